# revision 15
# baseline (speedup 1.0000x reference)
"""ConvGraphLayer kernel for 8 Trainium2 NeuronCores — fp8 DoubleRow version.

v3 baseline (measured 59.9us, rel 9.5e-3). Bisecting the v4/v5 regressions.
"""

import sys

import numpy as np

try:
    import concourse.bacc as bacc
except ImportError:  # concourse ships in the container image, not on PyPI
    for _p in ("/opt/trn_rl_repo", "/root/.axon_site/_ro/trn_rl_repo"):
        if _p not in sys.path:
            sys.path.append(_p)
    import concourse.bacc as bacc

import ml_dtypes
import concourse.mybir as mybir
import concourse.tile as tile
from concourse import bass_utils

N_NODES = 10000
F = 64
H = 64
EPS = 1e-7
N_CORES = 8
ROWS = N_NODES // N_CORES  # 1250 rows per core

F32 = mybir.dt.float32
F32R = mybir.dt.float32r
F8 = mybir.dt.float8e4
E4M3 = ml_dtypes.float8_e4m3
DR = mybir.MatmulPerfMode.DoubleRow

G3 = 12                       # z DMA groups of 3 k256-tiles
TPG = 3
G1 = 3                        # single-tile z groups at the end (short tail)
NGROUPS = G3 + G1
KT = G3 * TPG + G1            # 39 full 256-deep DoubleRow k-tiles
XHEAD = 6                     # k-tiles of x in the leading xq DMA
K_MAIN = KT * 256             # 9984
K_TAIL = N_NODES - K_MAIN + 1  # 16 leftover rows + 1 colsum row = 17
ICHUNKS = [(0, 512), (512, 482), (994, 256)]

TRACE = False
TRACE_KWARGS = {}
LAST_RESULTS = None

_PROGRAM = None


def _build_body(tc, nc, zq3, zq1, z_tail, xq, x_tail, x_selfT, nn_row,
                fc_w1, fc_w2, fc_b_col, out_rowsT):
    RELU = mybir.ActivationFunctionType.Relu
    COPY = mybir.ActivationFunctionType.Copy

    with (
        tc.tile_pool(name="const", bufs=1) as cpool,
        tc.tile_pool(name="stream3", bufs=5) as spool3,
        tc.tile_pool(name="stream1", bufs=3) as spool1,
        tc.tile_pool(name="psum", bufs=1, space="PSUM") as ppool,
    ):
        # x head first, then the first two z groups, so the PE starts early;
        # the rest of x is queued before z group 2 needs it
        xc_sb = cpool.tile([128, 2 * KT, 2 * F], F8, name="xc_sb", tag="xc_sb")
        nc.sync.dma_start(xc_sb[:, : 2 * XHEAD, :], xq[:, : 2 * XHEAD, :])

        zg_sbs = []

        def z_group_dma(g):
            if g < G3:
                zg = spool3.tile([128, 2 * TPG, ROWS], F8, name="zg3_sb", tag="zg3")
                nc.sync.dma_start(zg[:, :, :], zq3[g * 128 : (g + 1) * 128, :, :])
            else:
                s = g - G3
                zg = spool1.tile([128, 2, ROWS], F8, name="zg1_sb", tag="zg1")
                nc.sync.dma_start(zg[:, :, :], zq1[s * 128 : (s + 1) * 128, :, :])
            zg_sbs.append(zg)

        z_group_dma(0)
        z_group_dma(1)

        zt_tail = cpool.tile([K_TAIL, ROWS], F8, name="zt_tail", tag="zt_tail")
        nc.sync.dma_start(zt_tail[:, :], z_tail[:, :])
        xt_tail = cpool.tile([K_TAIL, 2 * F], F8, name="xt_tail", tag="xt_tail")
        nc.sync.dma_start(xt_tail[:, :], x_tail[:, :])
        nn_sb = cpool.tile([1, ROWS], F32, name="nn_sb", tag="nn_sb")
        nc.sync.dma_start(nn_sb[:, :], nn_row[:, :])
        fcw1_sb = cpool.tile([2 * F, H], F32R, name="fcw1_sb", tag="fcw1_sb")
        nc.sync.dma_start(fcw1_sb[:, :], fc_w1[:, :])
        fcw2_sb = cpool.tile([F, H], F32R, name="fcw2_sb", tag="fcw2_sb")
        nc.sync.dma_start(fcw2_sb[:, :], fc_w2[:, :])
        fcb_sb = cpool.tile([H, 1], F32, name="fcb_sb", tag="fcb_sb")
        nc.sync.dma_start(fcb_sb[:, :], fc_b_col[:, :])
        xself_sb = cpool.tile([F, ROWS], F32R, name="xself_sb", tag="xself_sb")
        nc.sync.dma_start(xself_sb[:, :], x_selfT[:, :])
        nc.sync.dma_start(xc_sb[:, 2 * XHEAD :, :], xq[:, 2 * XHEAD :, :])

        nn_eps = cpool.tile([1, ROWS], F32, name="nn_eps", tag="nn_eps")
        nc.scalar.activation(nn_eps[:, :], nn_sb[:, :], COPY, bias=2.0 * EPS, scale=2.0)
        recip = cpool.tile([1, ROWS], F32, name="recip", tag="recip")
        nc.vector.reciprocal(recip[:, :], nn_eps[:, :])
        ones_f = cpool.tile([1, 2 * F], F32, name="ones_f", tag="ones_f")
        nc.vector.memset(ones_f[:, :], 1.0)
        rc_ps = [
            ppool.tile([128, w], F32, name=f"rc_ps{ci}", tag=f"rc_ps{ci}")
            for ci, (_, w) in enumerate(ICHUNKS)
        ]
        recip_sb = cpool.tile([128, ROWS], F32, name="recip_sb", tag="recip_sb")

        nbscT = cpool.tile([128, ROWS], F32R, name="nbscT", tag="nbscT")

        nb_ps = [
            ppool.tile([128, w], F32, name=f"nb_ps{ci}", tag=f"nb_ps{ci}")
            for ci, (_, w) in enumerate(ICHUNKS)
        ]

        for g in range(NGROUPS):
            if g >= 2:
                z_group_dma(g)
            zg_sb = zg_sbs[g]
            for j in range(TPG if g < G3 else 1):
                kt = (g * TPG + j) if g < G3 else (G3 * TPG + (g - G3))
                lhs = xc_sb[:, 2 * kt : 2 * kt + 2, :]
                for ci, (o, w) in enumerate(ICHUNKS):
                    nc.tensor.matmul(
                        nb_ps[ci][:, :], lhs, zg_sb[:, 2 * j : 2 * j + 2, o : o + w],
                        start=(kt == 0), stop=(kt == KT - 1), perf_mode=DR,
                    )
            if g == 1:
                for ci, (o, w) in enumerate(ICHUNKS):
                    nc.tensor.matmul(
                        nb_ps[ci][:, :], xt_tail[:, :], zt_tail[:, o : o + w],
                        start=False, stop=False,
                    )
            if g == 8:
                for ci, (o, w) in enumerate(ICHUNKS):
                    nc.tensor.matmul(
                        rc_ps[ci][:, :], ones_f[:, :], recip[:, o : o + w],
                        start=True, stop=True,
                    )
                    nc.scalar.activation(
                        recip_sb[:, o : o + w], rc_ps[ci][:, :], COPY
                    )

        oT_ps = [
            ppool.tile([128, w], F32, name=f"oT_ps{ci}", tag=f"rc_ps{ci}")
            for ci, (_, w) in enumerate(ICHUNKS)
        ]
        outT_sb = cpool.tile([H, ROWS], F32, name="outT_sb", tag="outT_sb")
        for ci, (o, w) in enumerate(ICHUNKS):
            nc.vector.tensor_mul(
                nbscT[:, o : o + w], nb_ps[ci][:, :], recip_sb[:, o : o + w]
            )
            nc.tensor.matmul(
                oT_ps[ci][0:64, :], fcw1_sb[:, :], nbscT[:, o : o + w],
                start=True, stop=False,
            )
            nc.tensor.matmul(
                oT_ps[ci][0:64, :], fcw2_sb[:, :], xself_sb[:, o : o + w],
                start=False, stop=True,
            )
            nc.scalar.activation(
                outT_sb[:, o : o + w], oT_ps[ci][0:64, :], RELU, bias=fcb_sb[:, :]
            )
            nc.sync.dma_start(out_rowsT[:, o : o + w], outT_sb[:, o : o + w])


def _get_program():
    global _PROGRAM
    if _PROGRAM is not None:
        return _PROGRAM
    nc = bacc.Bacc("TRN2", target_bir_lowering=False, debug=False)
    zq3 = nc.dram_tensor("zq3", [G3 * 128, 2 * TPG, ROWS], F8, kind="ExternalInput").ap()
    zq1 = nc.dram_tensor("zq1", [G1 * 128, 2, ROWS], F8, kind="ExternalInput").ap()
    z_tail = nc.dram_tensor("z_tail", [K_TAIL, ROWS], F8, kind="ExternalInput").ap()
    xq = nc.dram_tensor("xq", [128, 2 * KT, 2 * F], F8, kind="ExternalInput").ap()
    x_tail = nc.dram_tensor("x_tail", [K_TAIL, 2 * F], F8, kind="ExternalInput").ap()
    x_selfT = nc.dram_tensor("x_selfT", [F, ROWS], F32R, kind="ExternalInput").ap()
    nn_row = nc.dram_tensor("nn_row", [1, ROWS], F32, kind="ExternalInput").ap()
    fc_w1 = nc.dram_tensor("fc_w1", [2 * F, H], F32R, kind="ExternalInput").ap()
    fc_w2 = nc.dram_tensor("fc_w2", [F, H], F32R, kind="ExternalInput").ap()
    fc_b_col = nc.dram_tensor("fc_b_col", [H, 1], F32, kind="ExternalInput").ap()
    out_rowsT = nc.dram_tensor("out_rowsT", [H, ROWS], F32, kind="ExternalOutput").ap()

    with tile.TileContext(nc) as tc:
        _build_body(tc, nc, zq3, zq1, z_tail, xq, x_tail, x_selfT, nn_row,
                    fc_w1, fc_w2, fc_b_col, out_rowsT)
    nc.compile()
    _PROGRAM = nc
    return nc


def _stage_inputs(x, adj_matrix, num_neighbors, fc_w, fc_b):
    x_hi = x.astype(E4M3)
    x_lo = (x - x_hi.astype(np.float32)).astype(E4M3)

    xcat = np.concatenate(
        [x_hi[:K_MAIN].reshape(KT, 2, 128, F), x_lo[:K_MAIN].reshape(KT, 2, 128, F)],
        axis=-1,
    )  # [KT, 2, 128, 2F]
    xq = np.ascontiguousarray(
        xcat.transpose(2, 0, 1, 3).reshape(128, 2 * KT, 2 * F)
    )

    colsum = (x.sum(axis=0, dtype=np.float64) / 4.0).astype(np.float32)
    xt_f32 = np.zeros((K_TAIL, F), dtype=np.float32)
    xt_f32[: K_TAIL - 1] = x[K_MAIN:]
    xt_f32[K_TAIL - 1] = colsum
    xt_hi = xt_f32.astype(E4M3)
    xt_lo = (xt_f32 - xt_hi.astype(np.float32)).astype(E4M3)
    x_tail = np.concatenate([xt_hi, xt_lo], axis=1)  # [K_TAIL, 2F]

    zT = (2.0 * adj_matrix.T - 1.0).astype(E4M3)  # [N (k), N (i)]

    xT = np.ascontiguousarray(x.T)  # [F, N]
    fc_w1 = np.ascontiguousarray(
        np.concatenate([fc_w[:, F:].T, fc_w[:, F:].T], axis=0)
    )  # [2F, H]
    fc_w2 = np.ascontiguousarray(fc_w[:, :F].T)  # [F, H]
    fc_b_col = np.ascontiguousarray(fc_b).reshape(H, 1)

    in_maps = []
    for c in range(N_CORES):
        sl = slice(c * ROWS, (c + 1) * ROWS)
        z_c = zT[:, sl]  # [N, ROWS]
        zq3_c = np.ascontiguousarray(
            z_c[: G3 * TPG * 256].reshape(G3, 2 * TPG, 128, ROWS).transpose(0, 2, 1, 3)
            .reshape(G3 * 128, 2 * TPG, ROWS)
        )
        zq1_c = np.ascontiguousarray(
            z_c[G3 * TPG * 256 : K_MAIN].reshape(G1, 2, 128, ROWS).transpose(0, 2, 1, 3)
            .reshape(G1 * 128, 2, ROWS)
        )
        z_tail = np.empty((K_TAIL, ROWS), dtype=E4M3)
        z_tail[: K_TAIL - 1] = z_c[K_MAIN:]
        z_tail[K_TAIL - 1] = np.float32(4.0)
        in_maps.append(
            {
                "zq3": zq3_c,
                "zq1": zq1_c,
                "z_tail": np.ascontiguousarray(z_tail),
                "xq": xq,
                "x_tail": np.ascontiguousarray(x_tail),
                "x_selfT": np.ascontiguousarray(xT[:, sl]),
                "nn_row": np.ascontiguousarray(num_neighbors[sl]).reshape(1, ROWS),
                "fc_w1": fc_w1,
                "fc_w2": fc_w2,
                "fc_b_col": fc_b_col,
            }
        )
    return in_maps


def kernel(x, adj_matrix, num_neighbors, fc_w, fc_b):
    global LAST_RESULTS
    x = np.ascontiguousarray(np.asarray(x, dtype=np.float32))
    adj_matrix = np.asarray(adj_matrix, dtype=np.float32)
    num_neighbors = np.asarray(num_neighbors, dtype=np.float32)
    fc_w = np.asarray(fc_w, dtype=np.float32)
    fc_b = np.asarray(fc_b, dtype=np.float32)
    assert adj_matrix.shape == (N_NODES, N_NODES)

    in_maps = _stage_inputs(x, adj_matrix, num_neighbors, fc_w, fc_b)

    nc = _get_program()
    results = bass_utils.run_bass_kernel_spmd(
        nc,
        in_maps,
        core_ids=list(range(N_CORES)),
        trace=TRACE,
        **TRACE_KWARGS,
    )
    LAST_RESULTS = results
    outs = [results.results[c]["out_rowsT"].T for c in range(N_CORES)]
    return np.ascontiguousarray(np.concatenate(outs, axis=0)).astype(
        np.float32, copy=False
    )


# revision 16
# speedup vs baseline: 1.0461x; 1.0461x over previous
"""ConvGraphLayer kernel for 8 Trainium2 NeuronCores — fp8 DoubleRow version.

v3 baseline (measured 59.9us, rel 9.5e-3). Bisecting the v4/v5 regressions.
"""

import sys

import numpy as np

try:
    import concourse.bacc as bacc
except ImportError:  # concourse ships in the container image, not on PyPI
    for _p in ("/opt/trn_rl_repo", "/root/.axon_site/_ro/trn_rl_repo"):
        if _p not in sys.path:
            sys.path.append(_p)
    import concourse.bacc as bacc

import ml_dtypes
import concourse.mybir as mybir
import concourse.tile as tile
from concourse import bass_utils

N_NODES = 10000
F = 64
H = 64
EPS = 1e-7
N_CORES = 8
ROWS = N_NODES // N_CORES  # 1250 rows per core

F32 = mybir.dt.float32
F32R = mybir.dt.float32r
F8 = mybir.dt.float8e4
E4M3 = ml_dtypes.float8_e4m3
DR = mybir.MatmulPerfMode.DoubleRow

G3 = 12                       # z DMA groups of 3 k256-tiles
TPG = 3
G1 = 3                        # single-tile z groups at the end (short tail)
NGROUPS = G3 + G1
KT = G3 * TPG + G1            # 39 full 256-deep DoubleRow k-tiles
XHEAD = 6                     # k-tiles of x in the leading xq DMA
K_MAIN = KT * 256             # 9984
K_TAIL = N_NODES - K_MAIN + 1  # 16 leftover rows + 1 colsum row = 17
ICHUNKS = [(0, 512), (512, 482), (994, 256)]

TRACE = False
TRACE_KWARGS = {}
LAST_RESULTS = None

_PROGRAM = None


def _build_body(tc, nc, zq3, zq1, z_tail, xq, x_tail, x_selfT, nn_row,
                fc_w1, fc_w2, fc_b_col, out_rowsT):
    RELU = mybir.ActivationFunctionType.Relu
    COPY = mybir.ActivationFunctionType.Copy

    with (
        tc.tile_pool(name="const", bufs=1) as cpool,
        tc.tile_pool(name="stream3", bufs=5) as spool3,
        tc.tile_pool(name="stream1", bufs=3) as spool1,
        tc.tile_pool(name="psum", bufs=1, space="PSUM") as ppool,
    ):
        # nn first (tiny; unblocks the recip chain early), then x, then the
        # first three z groups ahead of the other constants so the PE never
        # starves at group 2
        nn_sb = cpool.tile([1, ROWS], F32, name="nn_sb", tag="nn_sb")
        nc.sync.dma_start(nn_sb[:, :], nn_row[:, :])
        xc_sb = cpool.tile([128, 2 * KT, 2 * F], F8, name="xc_sb", tag="xc_sb")
        nc.sync.dma_start(xc_sb[:, :, :], xq[:, :, :])

        zg_sbs = []

        def z_group_dma(g):
            if g < G3:
                zg = spool3.tile([128, 2 * TPG, ROWS], F8, name="zg3_sb", tag="zg3")
                nc.sync.dma_start(zg[:, :, :], zq3[g * 128 : (g + 1) * 128, :, :])
            else:
                s = g - G3
                zg = spool1.tile([128, 2, ROWS], F8, name="zg1_sb", tag="zg1")
                nc.sync.dma_start(zg[:, :, :], zq1[s * 128 : (s + 1) * 128, :, :])
            zg_sbs.append(zg)

        z_group_dma(0)
        z_group_dma(1)
        z_group_dma(2)

        zt_tail = cpool.tile([K_TAIL, ROWS], F8, name="zt_tail", tag="zt_tail")
        nc.sync.dma_start(zt_tail[:, :], z_tail[:, :])
        xt_tail = cpool.tile([K_TAIL, 2 * F], F8, name="xt_tail", tag="xt_tail")
        nc.sync.dma_start(xt_tail[:, :], x_tail[:, :])
        fcw1_sb = cpool.tile([2 * F, H], F32R, name="fcw1_sb", tag="fcw1_sb")
        nc.sync.dma_start(fcw1_sb[:, :], fc_w1[:, :])
        fcw2_sb = cpool.tile([F, H], F32R, name="fcw2_sb", tag="fcw2_sb")
        nc.sync.dma_start(fcw2_sb[:, :], fc_w2[:, :])
        fcb_sb = cpool.tile([H, 1], F32, name="fcb_sb", tag="fcb_sb")
        nc.sync.dma_start(fcb_sb[:, :], fc_b_col[:, :])
        xself_sb = cpool.tile([F, ROWS], F32R, name="xself_sb", tag="xself_sb")
        nc.sync.dma_start(xself_sb[:, :], x_selfT[:, :])

        nn_eps = cpool.tile([1, ROWS], F32, name="nn_eps", tag="nn_eps")
        nc.scalar.activation(nn_eps[:, :], nn_sb[:, :], COPY, bias=2.0 * EPS, scale=2.0)
        ones_f = cpool.tile([1, 2 * F], F32, name="ones_f", tag="ones_f")
        nc.vector.memset(ones_f[:, :], 1.0)
        rc_ps = [
            ppool.tile([128, w], F32, name=f"rc_ps{ci}", tag=f"rc_ps{ci}")
            for ci, (_, w) in enumerate(ICHUNKS)
        ]
        recip_sb = cpool.tile([128, ROWS], F32, name="recip_sb", tag="recip_sb")

        nbscT = cpool.tile([128, ROWS], F32R, name="nbscT", tag="nbscT")

        nb_ps = [
            ppool.tile([128, w], F32, name=f"nb_ps{ci}", tag=f"nb_ps{ci}")
            for ci, (_, w) in enumerate(ICHUNKS)
        ]

        for g in range(NGROUPS):
            if g >= 3:
                z_group_dma(g)
            zg_sb = zg_sbs[g]
            for j in range(TPG if g < G3 else 1):
                kt = (g * TPG + j) if g < G3 else (G3 * TPG + (g - G3))
                lhs = xc_sb[:, 2 * kt : 2 * kt + 2, :]
                for ci, (o, w) in enumerate(ICHUNKS):
                    nc.tensor.matmul(
                        nb_ps[ci][:, :], lhs, zg_sb[:, 2 * j : 2 * j + 2, o : o + w],
                        start=(kt == 0), stop=(kt == KT - 1), perf_mode=DR,
                    )
            if g == 1:
                for ci, (o, w) in enumerate(ICHUNKS):
                    nc.tensor.matmul(
                        nb_ps[ci][:, :], xt_tail[:, :], zt_tail[:, o : o + w],
                        start=False, stop=False,
                    )
            if g == 8:
                # broadcast 2(nn+eps) to 128 partitions on the PE, then a
                # parallel 128-lane DVE reciprocal into recip_sb (the serial
                # [1, 1250] reciprocal costs ~8us and stalled the PE)
                for ci, (o, w) in enumerate(ICHUNKS):
                    nc.tensor.matmul(
                        rc_ps[ci][:, :], ones_f[:, :], nn_eps[:, o : o + w],
                        start=True, stop=True,
                    )
                    nc.vector.reciprocal(
                        recip_sb[:, o : o + w], rc_ps[ci][:, :]
                    )

        oT_ps = [
            ppool.tile([128, w], F32, name=f"oT_ps{ci}", tag=f"rc_ps{ci}")
            for ci, (_, w) in enumerate(ICHUNKS)
        ]
        outT_sb = cpool.tile([H, ROWS], F32, name="outT_sb", tag="outT_sb")
        for ci, (o, w) in enumerate(ICHUNKS):
            nc.vector.tensor_mul(
                nbscT[:, o : o + w], nb_ps[ci][:, :], recip_sb[:, o : o + w]
            )
            nc.tensor.matmul(
                oT_ps[ci][0:64, :], fcw1_sb[:, :], nbscT[:, o : o + w],
                start=True, stop=False,
            )
            nc.tensor.matmul(
                oT_ps[ci][0:64, :], fcw2_sb[:, :], xself_sb[:, o : o + w],
                start=False, stop=True,
            )
            nc.scalar.activation(
                outT_sb[:, o : o + w], oT_ps[ci][0:64, :], RELU, bias=fcb_sb[:, :]
            )
            nc.sync.dma_start(out_rowsT[:, o : o + w], outT_sb[:, o : o + w])


def _get_program():
    global _PROGRAM
    if _PROGRAM is not None:
        return _PROGRAM
    nc = bacc.Bacc("TRN2", target_bir_lowering=False, debug=False)
    zq3 = nc.dram_tensor("zq3", [G3 * 128, 2 * TPG, ROWS], F8, kind="ExternalInput").ap()
    zq1 = nc.dram_tensor("zq1", [G1 * 128, 2, ROWS], F8, kind="ExternalInput").ap()
    z_tail = nc.dram_tensor("z_tail", [K_TAIL, ROWS], F8, kind="ExternalInput").ap()
    xq = nc.dram_tensor("xq", [128, 2 * KT, 2 * F], F8, kind="ExternalInput").ap()
    x_tail = nc.dram_tensor("x_tail", [K_TAIL, 2 * F], F8, kind="ExternalInput").ap()
    x_selfT = nc.dram_tensor("x_selfT", [F, ROWS], F32R, kind="ExternalInput").ap()
    nn_row = nc.dram_tensor("nn_row", [1, ROWS], F32, kind="ExternalInput").ap()
    fc_w1 = nc.dram_tensor("fc_w1", [2 * F, H], F32R, kind="ExternalInput").ap()
    fc_w2 = nc.dram_tensor("fc_w2", [F, H], F32R, kind="ExternalInput").ap()
    fc_b_col = nc.dram_tensor("fc_b_col", [H, 1], F32, kind="ExternalInput").ap()
    out_rowsT = nc.dram_tensor("out_rowsT", [H, ROWS], F32, kind="ExternalOutput").ap()

    with tile.TileContext(nc) as tc:
        _build_body(tc, nc, zq3, zq1, z_tail, xq, x_tail, x_selfT, nn_row,
                    fc_w1, fc_w2, fc_b_col, out_rowsT)
    nc.compile()
    _PROGRAM = nc
    return nc


def _stage_inputs(x, adj_matrix, num_neighbors, fc_w, fc_b):
    x_hi = x.astype(E4M3)
    x_lo = (x - x_hi.astype(np.float32)).astype(E4M3)

    xcat = np.concatenate(
        [x_hi[:K_MAIN].reshape(KT, 2, 128, F), x_lo[:K_MAIN].reshape(KT, 2, 128, F)],
        axis=-1,
    )  # [KT, 2, 128, 2F]
    xq = np.ascontiguousarray(
        xcat.transpose(2, 0, 1, 3).reshape(128, 2 * KT, 2 * F)
    )

    colsum = (x.sum(axis=0, dtype=np.float64) / 4.0).astype(np.float32)
    xt_f32 = np.zeros((K_TAIL, F), dtype=np.float32)
    xt_f32[: K_TAIL - 1] = x[K_MAIN:]
    xt_f32[K_TAIL - 1] = colsum
    xt_hi = xt_f32.astype(E4M3)
    xt_lo = (xt_f32 - xt_hi.astype(np.float32)).astype(E4M3)
    x_tail = np.concatenate([xt_hi, xt_lo], axis=1)  # [K_TAIL, 2F]

    zT = (2.0 * adj_matrix.T - 1.0).astype(E4M3)  # [N (k), N (i)]

    xT = np.ascontiguousarray(x.T)  # [F, N]
    fc_w1 = np.ascontiguousarray(
        np.concatenate([fc_w[:, F:].T, fc_w[:, F:].T], axis=0)
    )  # [2F, H]
    fc_w2 = np.ascontiguousarray(fc_w[:, :F].T)  # [F, H]
    fc_b_col = np.ascontiguousarray(fc_b).reshape(H, 1)

    in_maps = []
    for c in range(N_CORES):
        sl = slice(c * ROWS, (c + 1) * ROWS)
        z_c = zT[:, sl]  # [N, ROWS]
        zq3_c = np.ascontiguousarray(
            z_c[: G3 * TPG * 256].reshape(G3, 2 * TPG, 128, ROWS).transpose(0, 2, 1, 3)
            .reshape(G3 * 128, 2 * TPG, ROWS)
        )
        zq1_c = np.ascontiguousarray(
            z_c[G3 * TPG * 256 : K_MAIN].reshape(G1, 2, 128, ROWS).transpose(0, 2, 1, 3)
            .reshape(G1 * 128, 2, ROWS)
        )
        z_tail = np.empty((K_TAIL, ROWS), dtype=E4M3)
        z_tail[: K_TAIL - 1] = z_c[K_MAIN:]
        z_tail[K_TAIL - 1] = np.float32(4.0)
        in_maps.append(
            {
                "zq3": zq3_c,
                "zq1": zq1_c,
                "z_tail": np.ascontiguousarray(z_tail),
                "xq": xq,
                "x_tail": np.ascontiguousarray(x_tail),
                "x_selfT": np.ascontiguousarray(xT[:, sl]),
                "nn_row": np.ascontiguousarray(num_neighbors[sl]).reshape(1, ROWS),
                "fc_w1": fc_w1,
                "fc_w2": fc_w2,
                "fc_b_col": fc_b_col,
            }
        )
    return in_maps


def kernel(x, adj_matrix, num_neighbors, fc_w, fc_b):
    global LAST_RESULTS
    x = np.ascontiguousarray(np.asarray(x, dtype=np.float32))
    adj_matrix = np.asarray(adj_matrix, dtype=np.float32)
    num_neighbors = np.asarray(num_neighbors, dtype=np.float32)
    fc_w = np.asarray(fc_w, dtype=np.float32)
    fc_b = np.asarray(fc_b, dtype=np.float32)
    assert adj_matrix.shape == (N_NODES, N_NODES)

    in_maps = _stage_inputs(x, adj_matrix, num_neighbors, fc_w, fc_b)

    nc = _get_program()
    results = bass_utils.run_bass_kernel_spmd(
        nc,
        in_maps,
        core_ids=list(range(N_CORES)),
        trace=TRACE,
        **TRACE_KWARGS,
    )
    LAST_RESULTS = results
    outs = [results.results[c]["out_rowsT"].T for c in range(N_CORES)]
    return np.ascontiguousarray(np.concatenate(outs, axis=0)).astype(
        np.float32, copy=False
    )


# revision 18
# speedup vs baseline: 1.0754x; 1.0281x over previous
"""ConvGraphLayer kernel for 8 Trainium2 NeuronCores — fp8 DoubleRow version.

v3 baseline (measured 59.9us, rel 9.5e-3). Bisecting the v4/v5 regressions.
"""

import sys

import numpy as np

try:
    import concourse.bacc as bacc
except ImportError:  # concourse ships in the container image, not on PyPI
    for _p in ("/opt/trn_rl_repo", "/root/.axon_site/_ro/trn_rl_repo"):
        if _p not in sys.path:
            sys.path.append(_p)
    import concourse.bacc as bacc

import ml_dtypes
import concourse.mybir as mybir
import concourse.tile as tile
from concourse import bass_utils

N_NODES = 10000
F = 64
H = 64
EPS = 1e-7
N_CORES = 8
ROWS = N_NODES // N_CORES  # 1250 rows per core

F32 = mybir.dt.float32
F32R = mybir.dt.float32r
F8 = mybir.dt.float8e4
E4M3 = ml_dtypes.float8_e4m3
DR = mybir.MatmulPerfMode.DoubleRow

G3 = 12                       # z DMA groups of 3 k256-tiles
TPG = 3
G1 = 3                        # single-tile z groups at the end (short tail)
NGROUPS = G3 + G1
KT = G3 * TPG + G1            # 39 full 256-deep DoubleRow k-tiles
XHEAD = 6                     # k-tiles of x in the leading xq DMA
K_MAIN = KT * 256             # 9984
K_TAIL = N_NODES - K_MAIN + 1  # 16 leftover rows + 1 colsum row = 17
ICHUNKS = [(0, 512), (512, 482), (994, 256)]

TRACE = False
TRACE_KWARGS = {}
LAST_RESULTS = None

_PROGRAM = None


def _build_body(tc, nc, zq3, zq1, z_tail, xq, x_tail, x_selfT, nn_row,
                fc_w1, fc_w2, fc_b_col, out_rowsT):
    RELU = mybir.ActivationFunctionType.Relu
    COPY = mybir.ActivationFunctionType.Copy

    with (
        tc.tile_pool(name="const", bufs=1) as cpool,
        tc.tile_pool(name="stream3", bufs=5) as spool3,
        tc.tile_pool(name="stream1", bufs=3) as spool1,
        tc.tile_pool(name="psum", bufs=1, space="PSUM") as ppool,
    ):
        # nn first (tiny; unblocks the recip chain early), then x, then the
        # first three z groups ahead of the other constants so the PE never
        # starves at group 2
        nn_sb = cpool.tile([1, ROWS], F32, name="nn_sb", tag="nn_sb")
        nc.sync.dma_start(nn_sb[:, :], nn_row[:, :])
        xc_sb = cpool.tile([128, 2 * KT, 2 * F], F8, name="xc_sb", tag="xc_sb")
        nc.sync.dma_start(xc_sb[:, :, :], xq[:, :, :])

        zg_sbs = []

        def z_group_dma(g):
            if g < G3:
                zg = spool3.tile([128, 2 * TPG, ROWS], F8, name="zg3_sb", tag="zg3")
                nc.sync.dma_start(zg[:, :, :], zq3[g * 128 : (g + 1) * 128, :, :])
            else:
                s = g - G3
                zg = spool1.tile([128, 2, ROWS], F8, name="zg1_sb", tag="zg1")
                nc.sync.dma_start(zg[:, :, :], zq1[s * 128 : (s + 1) * 128, :, :])
            zg_sbs.append(zg)

        z_group_dma(0)
        z_group_dma(1)
        z_group_dma(2)

        zt_tail = cpool.tile([K_TAIL, ROWS], F8, name="zt_tail", tag="zt_tail")
        nc.sync.dma_start(zt_tail[:, :], z_tail[:, :])
        xt_tail = cpool.tile([K_TAIL, 2 * F], F8, name="xt_tail", tag="xt_tail")
        nc.sync.dma_start(xt_tail[:, :], x_tail[:, :])
        fcw1_sb = cpool.tile([2 * F, H], F32R, name="fcw1_sb", tag="fcw1_sb")
        nc.sync.dma_start(fcw1_sb[:, :], fc_w1[:, :])
        fcw2_sb = cpool.tile([F, H], F32R, name="fcw2_sb", tag="fcw2_sb")
        nc.sync.dma_start(fcw2_sb[:, :], fc_w2[:, :])
        fcb_sb = cpool.tile([H, 1], F32, name="fcb_sb", tag="fcb_sb")
        nc.sync.dma_start(fcb_sb[:, :], fc_b_col[:, :])
        xself_sb = cpool.tile([F, ROWS], F32R, name="xself_sb", tag="xself_sb")
        nc.sync.dma_start(xself_sb[:, :], x_selfT[:, :])

        nn_eps = cpool.tile([1, ROWS], F32, name="nn_eps", tag="nn_eps")
        nc.scalar.activation(nn_eps[:, :], nn_sb[:, :], COPY, bias=2.0 * EPS, scale=2.0)
        ones_f = cpool.tile([1, 2 * F], F32, name="ones_f", tag="ones_f")
        nc.vector.memset(ones_f[:, :], 1.0)
        rc_ps = [
            ppool.tile([128, w], F32, name=f"rc_ps{ci}", tag=f"rc_ps{ci}")
            for ci, (_, w) in enumerate(ICHUNKS)
        ]
        recip_sb = cpool.tile([128, ROWS], F32, name="recip_sb", tag="recip_sb")

        nbscT = cpool.tile([128, ROWS], F32R, name="nbscT", tag="nbscT")

        nb_ps = [
            ppool.tile([128, w], F32, name=f"nb_ps{ci}", tag=f"nb_ps{ci}")
            for ci, (_, w) in enumerate(ICHUNKS)
        ]

        for g in range(NGROUPS):
            if g >= 3:
                z_group_dma(g)
            zg_sb = zg_sbs[g]
            last_group = g == NGROUPS - 1
            for j in range(TPG if g < G3 else 1):
                kt = (g * TPG + j) if g < G3 else (G3 * TPG + (g - G3))
                lhs = xc_sb[:, 2 * kt : 2 * kt + 2, :]
                # in the final tile, close the narrow chunk first so its
                # scale->FC->store chain overlaps the wide chunks' matmuls
                order = reversed(list(enumerate(ICHUNKS))) if last_group \
                    else enumerate(ICHUNKS)
                for ci, (o, w) in order:
                    nc.tensor.matmul(
                        nb_ps[ci][:, :], lhs, zg_sb[:, 2 * j : 2 * j + 2, o : o + w],
                        start=(kt == 0), stop=(kt == KT - 1), perf_mode=DR,
                    )
            if g == 1:
                for ci, (o, w) in enumerate(ICHUNKS):
                    nc.tensor.matmul(
                        nb_ps[ci][:, :], xt_tail[:, :], zt_tail[:, o : o + w],
                        start=False, stop=False,
                    )
            if g == 8:
                # broadcast 2(nn+eps) to 128 partitions on the PE, then a
                # parallel 128-lane DVE reciprocal into recip_sb (the serial
                # [1, 1250] reciprocal costs ~8us and stalled the PE)
                for ci, (o, w) in enumerate(ICHUNKS):
                    nc.tensor.matmul(
                        rc_ps[ci][:, :], ones_f[:, :], nn_eps[:, o : o + w],
                        start=True, stop=True,
                    )
                    nc.vector.reciprocal(
                        recip_sb[:, o : o + w], rc_ps[ci][:, :]
                    )

        oT_ps = [
            ppool.tile([128, w], F32, name=f"oT_ps{ci}", tag=f"rc_ps{ci}")
            for ci, (_, w) in enumerate(ICHUNKS)
        ]
        outT_sb = cpool.tile([H, ROWS], F32, name="outT_sb", tag="outT_sb")
        for ci, (o, w) in reversed(list(enumerate(ICHUNKS))):
            nc.vector.tensor_mul(
                nbscT[:, o : o + w], nb_ps[ci][:, :], recip_sb[:, o : o + w]
            )
            nc.tensor.matmul(
                oT_ps[ci][0:64, :], fcw1_sb[:, :], nbscT[:, o : o + w],
                start=True, stop=False,
            )
            nc.tensor.matmul(
                oT_ps[ci][0:64, :], fcw2_sb[:, :], xself_sb[:, o : o + w],
                start=False, stop=True,
            )
            nc.scalar.activation(
                outT_sb[:, o : o + w], oT_ps[ci][0:64, :], RELU, bias=fcb_sb[:, :]
            )
            nc.sync.dma_start(out_rowsT[:, o : o + w], outT_sb[:, o : o + w])


def _get_program():
    global _PROGRAM
    if _PROGRAM is not None:
        return _PROGRAM
    nc = bacc.Bacc("TRN2", target_bir_lowering=False, debug=False)
    zq3 = nc.dram_tensor("zq3", [G3 * 128, 2 * TPG, ROWS], F8, kind="ExternalInput").ap()
    zq1 = nc.dram_tensor("zq1", [G1 * 128, 2, ROWS], F8, kind="ExternalInput").ap()
    z_tail = nc.dram_tensor("z_tail", [K_TAIL, ROWS], F8, kind="ExternalInput").ap()
    xq = nc.dram_tensor("xq", [128, 2 * KT, 2 * F], F8, kind="ExternalInput").ap()
    x_tail = nc.dram_tensor("x_tail", [K_TAIL, 2 * F], F8, kind="ExternalInput").ap()
    x_selfT = nc.dram_tensor("x_selfT", [F, ROWS], F32R, kind="ExternalInput").ap()
    nn_row = nc.dram_tensor("nn_row", [1, ROWS], F32, kind="ExternalInput").ap()
    fc_w1 = nc.dram_tensor("fc_w1", [2 * F, H], F32R, kind="ExternalInput").ap()
    fc_w2 = nc.dram_tensor("fc_w2", [F, H], F32R, kind="ExternalInput").ap()
    fc_b_col = nc.dram_tensor("fc_b_col", [H, 1], F32, kind="ExternalInput").ap()
    out_rowsT = nc.dram_tensor("out_rowsT", [H, ROWS], F32, kind="ExternalOutput").ap()

    with tile.TileContext(nc) as tc:
        _build_body(tc, nc, zq3, zq1, z_tail, xq, x_tail, x_selfT, nn_row,
                    fc_w1, fc_w2, fc_b_col, out_rowsT)
    nc.compile()
    _PROGRAM = nc
    return nc


def _stage_inputs(x, adj_matrix, num_neighbors, fc_w, fc_b):
    x_hi = x.astype(E4M3)
    x_lo = (x - x_hi.astype(np.float32)).astype(E4M3)

    xcat = np.concatenate(
        [x_hi[:K_MAIN].reshape(KT, 2, 128, F), x_lo[:K_MAIN].reshape(KT, 2, 128, F)],
        axis=-1,
    )  # [KT, 2, 128, 2F]
    xq = np.ascontiguousarray(
        xcat.transpose(2, 0, 1, 3).reshape(128, 2 * KT, 2 * F)
    )

    colsum = (x.sum(axis=0, dtype=np.float64) / 4.0).astype(np.float32)
    xt_f32 = np.zeros((K_TAIL, F), dtype=np.float32)
    xt_f32[: K_TAIL - 1] = x[K_MAIN:]
    xt_f32[K_TAIL - 1] = colsum
    xt_hi = xt_f32.astype(E4M3)
    xt_lo = (xt_f32 - xt_hi.astype(np.float32)).astype(E4M3)
    x_tail = np.concatenate([xt_hi, xt_lo], axis=1)  # [K_TAIL, 2F]

    zT = (2.0 * adj_matrix.T - 1.0).astype(E4M3)  # [N (k), N (i)]

    xT = np.ascontiguousarray(x.T)  # [F, N]
    fc_w1 = np.ascontiguousarray(
        np.concatenate([fc_w[:, F:].T, fc_w[:, F:].T], axis=0)
    )  # [2F, H]
    fc_w2 = np.ascontiguousarray(fc_w[:, :F].T)  # [F, H]
    fc_b_col = np.ascontiguousarray(fc_b).reshape(H, 1)

    in_maps = []
    for c in range(N_CORES):
        sl = slice(c * ROWS, (c + 1) * ROWS)
        z_c = zT[:, sl]  # [N, ROWS]
        zq3_c = np.ascontiguousarray(
            z_c[: G3 * TPG * 256].reshape(G3, 2 * TPG, 128, ROWS).transpose(0, 2, 1, 3)
            .reshape(G3 * 128, 2 * TPG, ROWS)
        )
        zq1_c = np.ascontiguousarray(
            z_c[G3 * TPG * 256 : K_MAIN].reshape(G1, 2, 128, ROWS).transpose(0, 2, 1, 3)
            .reshape(G1 * 128, 2, ROWS)
        )
        z_tail = np.empty((K_TAIL, ROWS), dtype=E4M3)
        z_tail[: K_TAIL - 1] = z_c[K_MAIN:]
        z_tail[K_TAIL - 1] = np.float32(4.0)
        in_maps.append(
            {
                "zq3": zq3_c,
                "zq1": zq1_c,
                "z_tail": np.ascontiguousarray(z_tail),
                "xq": xq,
                "x_tail": np.ascontiguousarray(x_tail),
                "x_selfT": np.ascontiguousarray(xT[:, sl]),
                "nn_row": np.ascontiguousarray(num_neighbors[sl]).reshape(1, ROWS),
                "fc_w1": fc_w1,
                "fc_w2": fc_w2,
                "fc_b_col": fc_b_col,
            }
        )
    return in_maps


def kernel(x, adj_matrix, num_neighbors, fc_w, fc_b):
    global LAST_RESULTS
    x = np.ascontiguousarray(np.asarray(x, dtype=np.float32))
    adj_matrix = np.asarray(adj_matrix, dtype=np.float32)
    num_neighbors = np.asarray(num_neighbors, dtype=np.float32)
    fc_w = np.asarray(fc_w, dtype=np.float32)
    fc_b = np.asarray(fc_b, dtype=np.float32)
    assert adj_matrix.shape == (N_NODES, N_NODES)

    in_maps = _stage_inputs(x, adj_matrix, num_neighbors, fc_w, fc_b)

    nc = _get_program()
    results = bass_utils.run_bass_kernel_spmd(
        nc,
        in_maps,
        core_ids=list(range(N_CORES)),
        trace=TRACE,
        **TRACE_KWARGS,
    )
    LAST_RESULTS = results
    outs = [results.results[c]["out_rowsT"].T for c in range(N_CORES)]
    return np.ascontiguousarray(np.concatenate(outs, axis=0)).astype(
        np.float32, copy=False
    )
